# revision 1
# baseline (speedup 1.0000x reference)
"""Causal self-attention on 8 trn2 NeuronCores.

Sharding: DP4 (batch) x TP2 (head groups of 8). Core c -> batch c//2,
head group c%2. Each core computes qkv^T for its 512 channels, causal
attention for its 8 heads over all T=2048 queries, and a partial
projection y_partial = O_g @ W_proj[rows_g] (+ b_proj on group 0).
Host sums the two partials per batch and transposes (kernel emits y^T).

All matmuls run as float32r (full-rate fp32 on the PE). Attention is
computed in the S^T = K Q^T orientation so softmax reduction lands on
the matmul contraction axis: row-sums come from a ones-column appended
to V, no max-subtraction (scores ~ N(0,1), exp can't overflow).
"""
import sys

sys.path.insert(0, "/opt/trn_rl_repo")

import numpy as np

import concourse.bass as bass
import concourse.tile as tile
from concourse import bacc, mybir

f32 = mybir.dt.float32
f32r = mybir.dt.float32r
AFT = mybir.ActivationFunctionType

N_CORES = 8
B, T, C = 4, 2048, 1024
H, HD = 16, 64            # total heads, head dim
HPC = 8                   # heads per core
CPC = 512                 # channels per core (q, k or v)
NT = T // 128             # 16 t-tiles of 128
NS = T // 512             # 4 t-slices of 512
NC_T = C // 128           # 8 C-tiles (contraction)
SCALE = 1.0 / np.sqrt(HD)


def build_nc(repeat: int = 1):
    """Build the per-core SPMD program. repeat>1 wraps the whole body in a
    dynamic loop (used only for timing amortization)."""
    nc = bacc.Bacc("TRN2", target_bir_lowering=False, debug=False,
                   num_devices=N_CORES)

    xb_d = nc.dram_tensor("xb", [C, T], f32, kind="ExternalInput")
    wqkv_d = nc.dram_tensor("wqkv", [C, 3 * CPC], f32, kind="ExternalInput")
    bqkv_d = nc.dram_tensor("bqkv", [128, 12], f32, kind="ExternalInput")
    wp_d = nc.dram_tensor("wp", [CPC, C], f32, kind="ExternalInput")
    bp_d = nc.dram_tensor("bp", [128, 8], f32, kind="ExternalInput")
    masks_d = nc.dram_tensor("masks", [128, 4 * 512], f32, kind="ExternalInput")
    yt_d = nc.dram_tensor("yT", [C, T], f32, kind="ExternalOutput")

    with tile.TileContext(nc) as tc:
        def body(_=None):
            _build_body(nc, tc, xb_d, wqkv_d, bqkv_d, wp_d, bp_d,
                        masks_d, yt_d)
        if repeat == 1:
            body()
        else:
            with tc.For_i(0, repeat, 1):
                body()
    nc.compile()
    return nc


def _build_body(nc, tc, xb_d, wqkv_d, bqkv_d, wp_d, bp_d, masks_d,
                yt_d):
    # ---------- persistent tiles (live through attention) ----------
    pers_cm = tc.tile_pool(name="pers", bufs=1)
    pers = pers_cm.__enter__()
    masks = pers.tile([128, 4 * 512], f32r, name="masks")
    nc.sync.dma_start(masks[:], masks_d.ap().bitcast(f32r))
    bqkv = pers.tile([128, 12], f32, name="bqkv")
    nc.sync.dma_start(bqkv[:], bqkv_d.ap())
    bp = pers.tile([128, 8], f32, name="bp")
    nc.sync.dma_start(bp[:], bp_d.ap())

    # qkv^T results: QT/KT [c=128 x 4 tiles, t=2048], V natural+ones
    qt = [pers.tile([128, T], f32r, name=f"qt{i}") for i in range(4)]
    kt = [pers.tile([128, T], f32r, name=f"kt{i}") for i in range(4)]
    vaug = [pers.tile([128, 8 * 65], f32r, name=f"vaug{i}") for i in range(NT)]
    for i in range(NT):
        # fill with 1.0; V copies overwrite cols 0-63 of each 65-group,
        # leaving the ones column (col 64) for the row-sum trick
        nc.gpsimd.memset(vaug[i][:].bitcast(f32), 1.0)

    # ---------- phase A: transpose x + qkv^T matmuls ----------
    with tc.tile_pool(name="wq", bufs=1) as wq_pool, \
         tc.tile_pool(name="xt", bufs=16) as xt_pool, \
         tc.tile_pool(name="pqk", bufs=3, space="PSUM") as pqk_pool, \
         tc.tile_pool(name="pv", bufs=3, space="PSUM") as pv_pool:

        wqkv = [wq_pool.tile([128, 3 * CPC], f32r, name=f"wqkv{ci}")
                for ci in range(NC_T)]
        for ci in range(NC_T):
            nc.sync.dma_start(wqkv[ci][:],
                              xb_slice_rows(wqkv_d, ci).bitcast(f32r))

        for s in range(NS):            # t-slices of 512
            # x^T comes pre-transposed from the host: DMA slice tiles
            xts = []
            for ci in range(NC_T):
                xtt = xt_pool.tile([128, 512], f32r, name="xt")
                nc.sync.dma_start(
                    xtt[:],
                    xb_d.ap()[128 * ci:128 * ci + 128,
                              512 * s:512 * s + 512].bitcast(f32r))
                xts.append(xtt)

            # Q/K: out[c_out 128, t 512] = sum_ci wqkv[ci][:,cols].T @ xT[ci]
            for g in range(8):         # 0-3 Q tiles, 4-7 K tiles
                ps = pqk_pool.tile([128, 512], f32, name="pqk")
                for ci in range(NC_T):
                    nc.tensor.matmul(
                        ps[:], wqkv[ci][:, 128 * g:128 * g + 128], xts[ci][:],
                        start=(ci == 0), stop=(ci == NC_T - 1))
                dst = qt[g] if g < 4 else kt[g - 4]
                bias = bqkv[:, g:g + 1]
                scale = SCALE if g < 4 else 1.0
                nc.scalar.activation(dst[:, 512 * s:512 * s + 512], ps[:],
                                     AFT.Identity, bias=bias, scale=scale)

            # V: out[t 128, c_v 512] = sum_ci xT[ci][:, t128].T @ wqkv[ci][:, 1024:]
            for tt in range(4):
                ti = 4 * s + tt
                ps = pv_pool.tile([128, 512], f32, name="pv")
                for ci in range(NC_T):
                    nc.tensor.matmul(
                        ps[:], xts[ci][:, 128 * tt:128 * tt + 128],
                        wqkv[ci][:, 1024:1536],
                        start=(ci == 0), stop=(ci == NC_T - 1))
                dst = vaug[ti][:].rearrange("p (h w) -> p h w", w=65)[:, :, 0:64]
                nc.vector.tensor_copy(dst, ps[:].rearrange("p (h w) -> p h w", w=64))

    # ---------- phase B: attention ----------
    ot_cm = tc.tile_pool(name="otp", bufs=1)
    ot_p = ot_cm.__enter__()
    ot = [ot_p.tile([128, T], f32r, name=f"ot{i}") for i in range(4)]

    with tc.tile_pool(name="pt", bufs=4) as pt_pool, \
         tc.tile_pool(name="rl", bufs=4) as rl_pool, \
         tc.tile_pool(name="rlb", bufs=4) as rlb_pool, \
         tc.tile_pool(name="pst", bufs=2, space="PSUM") as pst_pool, \
         tc.tile_pool(name="pot", bufs=4, space="PSUM") as pot_pool:

        for hp in range(4):            # head pairs (2hp, 2hp+1)
            for jp in range(2):        # q-tile pairs {2jp, 2jp+1}
                j_list = [2 * jp, 2 * jp + 1]
                i_max = 4 * j_list[-1] + 3
                ots = {}               # (h_local, j) -> psum tile [65, 512]
                for hl in range(2):
                    for j in j_list:
                        ots[(hl, j)] = pot_pool.tile([65, 512], f32, name="pot")
                for i in range(i_max + 1):
                    vjs = [j for j in j_list if 128 * i <= 512 * j + 511]
                    nq = len(vjs)
                    for hl in range(2):
                        h = 2 * hp + hl
                        rows = slice(64 * hl, 64 * hl + 64)
                        st = pst_pool.tile([128, 1024], f32, name="pst")
                        for idx, j in enumerate(vjs):
                            nc.tensor.matmul(
                                st[:, 512 * idx:512 * idx + 512],
                                kt[hp][rows, 128 * i:128 * i + 128],
                                qt[hp][rows, 512 * j:512 * j + 512],
                                start=True, stop=True)
                        ptile = pt_pool.tile([128, 1024], f32r, name="pt")
                        nc.scalar.activation(ptile[:, :512 * nq],
                                             st[:, :512 * nq], AFT.Exp)
                        jd = i // 4    # diagonal q-tile for this k-block
                        if jd in vjs:
                            o = i % 4
                            idx = vjs.index(jd)
                            sub = ptile[:, 512 * idx:512 * idx + 512]
                            nc.vector.tensor_mul(
                                sub, sub, masks[:, 512 * o:512 * o + 512])
                        for idx, j in enumerate(vjs):
                            nc.tensor.matmul(
                                ots[(hl, j)][:],
                                vaug[i][:, 65 * h:65 * h + 65],
                                ptile[:, 512 * idx:512 * idx + 512],
                                start=(i == 0), stop=(i == 4 * j + 3))
                # normalize + v-bias, write O^T
                for hl in range(2):
                    h = 2 * hp + hl
                    rows = slice(64 * hl, 64 * hl + 64)
                    bv = bqkv[64 * hl:64 * hl + 64, 8 + hp:9 + hp]
                    for j in j_list:
                        po = ots[(hl, j)]
                        rl = rl_pool.tile([1, 512], f32, name="rl")
                        nc.vector.reciprocal(rl[:], po[64:65, :])
                        rlb = rlb_pool.tile([64, 512], f32, name="rlb")
                        nc.gpsimd.partition_broadcast(rlb[:], rl[:])
                        dst = ot[hp][rows, 512 * j:512 * j + 512]
                        nc.vector.tensor_mul(dst, po[0:64, :], rlb[:])
                        nc.vector.tensor_scalar_add(dst, dst, bv)

    # ---------- phase C: projection ----------
    with tc.tile_pool(name="wp", bufs=1) as wp_pool, \
         tc.tile_pool(name="yt", bufs=4) as yt_pool, \
         tc.tile_pool(name="py", bufs=2, space="PSUM") as py_pool:
        wp = [wp_pool.tile([128, C], f32r, name=f"wp{i}") for i in range(4)]
        for ci in range(4):
            nc.sync.dma_start(wp[ci][:],
                              wp_d.ap()[128 * ci:128 * ci + 128, :].bitcast(f32r))
        for g in range(8):             # output channel tiles
            for s in range(NS):
                ps = py_pool.tile([128, 512], f32, name="py")
                for ci in range(4):
                    nc.tensor.matmul(
                        ps[:], wp[ci][:, 128 * g:128 * g + 128],
                        ot[ci][:, 512 * s:512 * s + 512],
                        start=(ci == 0), stop=(ci == 3))
                yt = yt_pool.tile([128, 512], f32, name="yt")
                nc.scalar.activation(yt[:], ps[:], AFT.Identity,
                                     bias=bp[:, g:g + 1])
                nc.sync.dma_start(
                    yt_d.ap()[128 * g:128 * g + 128, 512 * s:512 * s + 512],
                    yt[:])

    ot_cm.__exit__(None, None, None)
    pers_cm.__exit__(None, None, None)


def xb_slice_rows(wqkv_d, ci):
    return wqkv_d.ap()[128 * ci:128 * ci + 128, :]


def make_inputs(x, W_attn, b_attn, W_proj, b_proj):
    """Host-side sharding: per-core input dicts."""
    x = np.asarray(x, np.float32)
    W_attn = np.asarray(W_attn, np.float32)
    b_attn = np.asarray(b_attn, np.float32)
    W_proj = np.asarray(W_proj, np.float32)
    b_proj = np.asarray(b_proj, np.float32)

    ident = np.eye(128, dtype=np.float32)
    # masks[kk, 512*o + qq] = 1 if kk + 128*o <= qq
    masks = np.zeros((128, 4 * 512), np.float32)
    kk = np.arange(128)[:, None]
    qq = np.arange(512)[None, :]
    for o in range(4):
        masks[:, 512 * o:512 * (o + 1)] = (kk + 128 * o <= qq)

    in_maps = []
    for core in range(N_CORES):
        b, g = divmod(core, 2)
        cols = np.concatenate([
            np.arange(CPC * g, CPC * g + CPC),
            C + np.arange(CPC * g, CPC * g + CPC),
            2 * C + np.arange(CPC * g, CPC * g + CPC)])
        wqkv = np.ascontiguousarray(W_attn[:, cols])
        bq = b_attn[cols].copy()                      # [1536]
        bq[:CPC] *= SCALE                             # fold q-scale into bias
        bqkv = np.ascontiguousarray(bq.reshape(12, 128).T)
        wp = np.ascontiguousarray(W_proj[CPC * g:CPC * g + CPC, :])
        bp = (b_proj if g == 0 else np.zeros(C, np.float32))
        bp = np.ascontiguousarray(bp.reshape(8, 128).T)
        in_maps.append({
            "xb": np.ascontiguousarray(x[b].T),
            "wqkv": wqkv,
            "bqkv": bqkv,
            "wp": wp,
            "bp": bp,
            "ident": ident,
            "masks": masks,
        })
    return in_maps


def unshard(results):
    """Combine per-core yT partials into [B, T, C] output."""
    out = np.empty((B, T, C), np.float32)
    for b in range(B):
        yt = results[2 * b]["yT"] + results[2 * b + 1]["yT"]
        out[b] = yt.T
    return out


_nc_cache = {}


def kernel(x, W_attn, b_attn, W_proj, b_proj):
    from concourse.bass_utils import run_bass_kernel_spmd
    if "nc" not in _nc_cache:
        _nc_cache["nc"] = build_nc(repeat=1)
    nc = _nc_cache["nc"]
    in_maps = make_inputs(x, W_attn, b_attn, W_proj, b_proj)
    res = run_bass_kernel_spmd(nc, in_maps, core_ids=list(range(N_CORES)),
                               trace=False)
    return unshard(res.results)

